# revision 7
# baseline (speedup 1.0000x reference)
"""Bahdanau additive attention on Trainium2 (Bass/Tile), SPMD over 8 NeuronCores.

Problem: attn_out[b,t,:] = softmax_s(v . tanh(enc_f[b,s,:] + qry_f[b,t,:])) @ enc[b]
  with enc_f = enc @ W_h^T, qry_f = q @ W_s^T, masked to s < src_lengths[b].

Sharding: parallel over tgt_len T - core i handles query rows [i*32,(i+1)*32)
for ALL batches; weights and encoder outputs replicated. Every core does the
same amount of work (all batches, full valid src range) so load is perfectly
balanced despite variable src_lengths.

Layout: hidden dim h lives on SBUF partitions (4 chunks of 128). For a query
row t, tanh(enc_fT[h,s] + qry_fT[h,t]) is a per-partition broadcast add + tanh,
and the reduction over h is a v-stationary matmul.

The tanh work (the arithmetic bottleneck; ACT runs at 1 elem/cycle/lane) is
split across THREE engines:
  - chunks 0,1 (+ even query rows of chunk 2): Pool does the broadcast add,
    ACT does an exact tanh (with the 1/gamma prescale folded into the
    activation's free scale field).
  - chunk 3 (+ odd query rows of chunk 2): a custom DVE uop (TANH3_ANT)
    computes clamp-cubic tanh with the bias add fused:
        t = clamp(x + qf, -B', B');  out = t + t^3*c3'
    in one 7-stage pass. The cubic's linear coefficient is folded into a
    global prescale gamma of W_h/W_s (applied on the host), so enc_f/qry_f
    arrive pre-scaled and the DVE op needs only one streamed coefficient.

src_lengths are read on the host at trace time: loop extents are specialized
to L_b (padded to a multiple of 4); masked source positions never computed.
"""

import numpy as np

NCORES = 8
P = 128

# clamp-cubic tanh fit: tanh(x) ~= u*(C1 + C3*u^2), u = clip(x, -BCLAMP, BCLAMP)
# (minimax on [0, inf), max abs err 0.0414)
TANH_C1 = 0.86775672
TANH_C3 = -0.10534837
TANH_B = 1.65700743
GAMMA = TANH_C1                      # global prescale folded into W_h/W_s
BCL = TANH_B * GAMMA                 # clamp bound in prescaled units
C3S = TANH_C3 / (GAMMA ** 3)         # cubic coeff in prescaled units

# Per chunk c (of 4), how many of each sweep's 16 query rows get the DVE
# cubic path (the rest go Pool-add + ACT-exact-tanh). Total DVE fraction =
# sum/64 of the hidden dim.
KDVE = (0, 0, 4, 16)


def _register_tanh3():
    """Idempotently register the fused clamp-cubic-tanh custom DVE op."""
    import concourse.dve_ops as dve_ops_mod
    from concourse.dve_ops import DveOp
    from concourse.dve_spec import (
        Spec, Src0, Src1, C0, C1, C2, minn, maxx, sq, lower, _has_src1,
    )
    from concourse.dve_uop import DveOpSpec

    name = "TANH3_ANT"
    for op in dve_ops_mod.OPS:
        if op.name == name:
            return op

    def _ref(in0, in1, s0, s1, imm2):
        t = np.clip(in0.astype(np.float32) + s0, s1, imm2)
        return (t + t * t * t * in1).astype(np.float32)

    _t = minn(maxx(Src0 + C0, C1), C2)
    spec = Spec(body=_t + (sq(_t) * _t) * Src1, reference=_ref)
    row = dve_ops_mod._CUSTOM_DVE_ROW_BASE + len(dve_ops_mod.OPS)
    shas = {}
    for ver in ("v3", "v4"):
        s = DveOpSpec(name=name, opcode=row, uops=lower(spec, ver=ver),
                      rd1_en=_has_src1(spec))
        shas[ver] = s.sha(ver)
    op = DveOp(name, spec, subdim=False, uops_sha=shas)
    dve_ops_mod.OPS.append(op)
    dve_ops_mod._SUB_OPCODE_FOR_NAME[name] = row
    dve_ops_mod.CUSTOM_DVE_SPECS[name] = spec
    return op


def _build_program(B, T_core, S, H, L, Lh, reps=1):
    import concourse.bass as bass  # noqa: F401
    import concourse.mybir as mybir
    import concourse.tile as tile
    from concourse import bacc

    TANH3 = _register_tanh3()

    f32 = mybir.dt.float32
    bf16 = mybir.dt.bfloat16
    AF = mybir.ActivationFunctionType

    HC = H // P  # h chunks (4)

    nc = bacc.Bacc("TRN2", target_bir_lowering=False, debug=False)

    enc_d = nc.declare_dram_parameter("enc", [B, S, H], f32, isOutput=False)
    q_d = nc.declare_dram_parameter("q", [B, T_core, H], f32, isOutput=False)
    # host-prepared: gamma-scaled, pre-transposed, chunk-blocked, bf16
    whT_d = nc.declare_dram_parameter("whT", [P, HC * H], bf16, isOutput=False)
    wsT_d = nc.declare_dram_parameter("wsT", [P, HC * H], bf16, isOutput=False)
    v32_d = nc.declare_dram_parameter("v32", [P, HC * 32], bf16, isOutput=False)
    out_d = nc.declare_dram_parameter("out", [B, T_core, H], f32, isOutput=True)

    with tile.TileContext(nc) as tc:
        with (
            tc.tile_pool(name="const", bufs=1) as constp,
            tc.tile_pool(name="sb", bufs=2) as sb,
            tc.tile_pool(name="work", bufs=2) as workp,
            tc.tile_pool(name="ps", bufs=2, space="PSUM") as psp,
            tc.tile_pool(name="ps_sc", bufs=1, space="PSUM") as pssc,
        ):
            from concourse.masks import make_identity
            ident_f = constp.tile([P, P], f32)
            make_identity(nc, ident_f)

            whT = constp.tile([P, HC * H], bf16)
            nc.sync.dma_start(whT, whT_d[:, :])
            wsT = constp.tile([P, HC * H], bf16)
            nc.sync.dma_start(wsT, wsT_d[:, :])
            v32 = constp.tile([P, HC * 32], bf16)
            nc.sync.dma_start(v32, v32_d[:, :])
            c3t = constp.tile([P, S], bf16)
            nc.gpsimd.memset(c3t, C3S)

            def load(b):
                """DMA the encoder rows (valid range only) + query slice."""
                Lhb = Lh[b]
                nk = (Lhb + P - 1) // P
                enc_nat = []
                for k2 in range(nk):
                    r2 = min(P, Lhb - k2 * P)
                    en = sb.tile([P, H], f32, name=f"enc{b}_{k2}", tag=f"enc{k2}", bufs=3)
                    nc.sync.dma_start(en[:r2, :], enc_d[b, k2 * P : k2 * P + r2, :])
                    enc_nat.append((en, r2))
                qn = sb.tile([T_core, H], f32, name=f"qn{b}", tag="qn", bufs=3)
                nc.sync.dma_start(qn, q_d[b])
                return enc_nat, qn

            def phase_a(b, enc_nat, qn):
                """encT, enc_fT' (bf16) and qry_fT' (f32) for batch b."""
                Lhb = Lh[b]
                nk = (Lhb + P - 1) // P
                # encT (f32): block k = enc^T[h' in chunk k, s]
                encT = sb.tile([P, HC * S], bf16, name=f"encT{b}", tag="encT", bufs=2)
                encT_v = encT.rearrange("p (k s) -> p k s", k=HC)
                for k2 in range(nk):
                    en, r2 = enc_nat[k2]
                    ps_t = psp.tile(
                        [P, HC * P], f32, name=f"encT_ps{b}_{k2}", tag="mmA", bufs=1
                    )
                    for k in range(HC):
                        nc.tensor.transpose(
                            ps_t[:, k * P : k * P + r2],
                            en[:r2, k * P : (k + 1) * P],
                            ident_f[:r2, :r2],
                        )
                    nc.vector.tensor_copy(
                        encT_v[:, :, k2 * P : k2 * P + r2],
                        ps_t.rearrange("p (k s) -> p k s", k=HC)[:, :, :r2],
                    )
                # enc_fT' (bf16): block c = gamma*(W_h @ enc^T)[h in chunk c, s]
                ps_e = psp.tile([P, HC * S], f32, name=f"encf_ps{b}", tag="mmA", bufs=1)
                for c in range(HC):
                    for k in range(HC):
                        nc.tensor.matmul(
                            ps_e[:, c * S : c * S + Lhb],
                            whT[:, k * H + c * P : k * H + (c + 1) * P],
                            encT_v[:, k, :Lhb],
                            start=(k == 0),
                            stop=(k == HC - 1),
                        )
                encfT = sb.tile([P, HC * S], bf16, name=f"encfT{b}", tag="encfT", bufs=3)
                nc.vector.tensor_copy(
                    encfT.rearrange("p (c s) -> p c s", c=HC)[:, :, :Lhb],
                    ps_e.rearrange("p (c s) -> p c s", c=HC)[:, :, :Lhb],
                )
                # qry_fT' (f32): block c cols = gamma*(W_s @ q^T)[h chunk c, t]
                ps_q = psp.tile([P, HC * T_core], f32, name=f"qT_ps{b}", tag="mmA", bufs=1)
                for k in range(HC):
                    nc.tensor.transpose(
                        ps_q[:, k * T_core : (k + 1) * T_core],
                        qn[:, k * P : (k + 1) * P],
                        ident_f[:T_core, :T_core],
                    )
                qT = sb.tile([P, HC * T_core], bf16, name=f"qT{b}", tag="qT", bufs=2)
                nc.vector.tensor_copy(qT, ps_q)
                ps_qf = psp.tile([P, HC * T_core], f32, name=f"qf_ps{b}", tag="mmA", bufs=1)
                for c in range(HC):
                    for k in range(HC):
                        nc.tensor.matmul(
                            ps_qf[:, c * T_core : (c + 1) * T_core],
                            wsT[:, k * H + c * P : k * H + (c + 1) * P],
                            qT[:, k * T_core : (k + 1) * T_core],
                            start=(k == 0),
                            stop=(k == HC - 1),
                        )
                qfT = sb.tile([P, HC * T_core], f32, name=f"qfT{b}", tag="qfT", bufs=3)
                nc.vector.tensor_copy(qfT, ps_qf)
                return encfT, qfT

            def phase_bc(b, enc_nat, encfT, qfT):
                Lb, Lhb = L[b], Lh[b]
                nk = (Lhb + P - 1) // P
                n_sweeps = T_core // 16

                sc_w = sb.tile([T_core, S], f32, name=f"scores{b}", tag="scsb", bufs=2)

                for sweep in range(n_sweeps):
                    # ---- tanh tiles for 16 query rows x 4 chunks ----
                    tanh_tiles = []
                    for c in range(HC):
                        kdve = KDVE[c]
                        kact = 16 - kdve
                        tanh_t = workp.tile(
                            [P, 16 * Lhb], bf16, name=f"tanh{b}_{sweep}_{c}",
                            tag=f"tanh{c}", bufs=2,
                        )
                        tanh_v = tanh_t.rearrange("p (t s) -> p t s", t=16)
                        if kact > 0:
                            # ACT rows: Pool broadcast-add (packed), one ACT
                            # tanh with the 1/gamma prescale in `scale`.
                            sum_t = workp.tile(
                                [P, kact * Lhb], bf16, name=f"sum{b}_{sweep}_{c}",
                                tag=f"sum{c}", bufs=2,
                            )
                            for i in range(kact):
                                tg = sweep * 16 + i
                                nc.gpsimd.tensor_scalar_add(
                                    sum_t[:, i * Lhb : (i + 1) * Lhb],
                                    encfT[:, c * S : c * S + Lhb],
                                    qfT[:, c * T_core + tg : c * T_core + tg + 1],
                                )
                            nc.scalar.activation(
                                tanh_v[:, :kact, :],
                                sum_t.rearrange("p (t s) -> p t s", t=kact),
                                AF.Tanh, scale=1.0 / GAMMA,
                            )
                        for i in range(kdve):
                            # DVE rows: fused bias-add + clamp-cubic tanh.
                            tg = sweep * 16 + kact + i
                            nc.vector._custom_dve(
                                TANH3,
                                out=tanh_v[:, kact + i, :],
                                in0=encfT[:, c * S : c * S + Lhb],
                                in1=c3t[:, :Lhb],
                                s0=qfT[:, c * T_core + tg : c * T_core + tg + 1],
                                s1=-BCL, imm2=BCL,
                            )
                        tanh_tiles.append(tanh_t)

                    # ---- scores: v-stationary matmuls, c-outer so each
                    # (c, colgroup) LDWEIGHTS serves 4 matmuls ----
                    ps_scores = pssc.tile(
                        [P, 2 * 512], f32, name=f"sc_ps{b}_{sweep}",
                        tag="scores", bufs=2,
                    )
                    # per query row: 4 consecutive chunk-matmuls (PSUM
                    # accumulation groups must not interleave within a bank)
                    for tt in range(16):
                        cg, m = tt // 4, tt % 4
                        for c in range(HC):
                            nc.tensor.matmul(
                                ps_scores[32 * cg : 32 * cg + 32,
                                          256 * m : 256 * m + Lhb],
                                v32[:, c * 32 : (c + 1) * 32],
                                tanh_tiles[c][:, tt * Lhb : (tt + 1) * Lhb],
                                start=(c == 0),
                                stop=(c == HC - 1),
                                tile_position=(0, 32 * cg),
                            )
                    # PSUM -> SBUF staging copy (partition-preserving), then
                    # a gather DMA to softmax layout: row (4*cg+m) <-
                    # stage[32*cg, 256*m : 256*m+Lb]
                    stage = sb.tile(
                        [P, 4 * 256], f32, name=f"stage{b}_{sweep}", tag="stage"
                    )
                    nc.vector.tensor_copy(
                        stage.rearrange("p (m s) -> p m s", m=4)[:, :, :Lb],
                        ps_scores.rearrange("p (m s) -> p m s", m=4)[:, :, :Lb],
                    )
                    src = stage.rearrange(
                        "(a p) (m s) -> a p m s", a=4, m=4
                    )[:, 0, :, :Lb]
                    nc.sync.dma_start(
                        sc_w[sweep * 16 : (sweep + 1) * 16, :Lb], src
                    )

                # ---- per-batch softmax over [T_core, Lb]; scores are
                # bounded (|score| <= ||v||*sqrt(H) ~ 23) so raw exp is safe
                # in f32 and the max-subtraction is skipped entirely ----
                w_sw = sb.tile([T_core, S], f32, name=f"w{b}", tag="w")
                if Lb < S:
                    nc.gpsimd.memset(w_sw[:, Lb:], 0.0)
                sums = sb.tile([T_core, 1], f32, name=f"sums{b}", tag="sums")
                nc.scalar.activation(
                    w_sw[:, :Lb], sc_w[:, :Lb],
                    AF.Exp, accum_out=sums,
                )
                recip = sb.tile([T_core, 1], f32, name=f"recip{b}", tag="recip")
                nc.vector.reciprocal(recip, sums)

                # attn_out = (w_raw @ enc) * recip
                ps_w = psp.tile([P, 2 * T_core], f32, name=f"wT_ps{b}", tag="mmC", bufs=2)
                for k2 in range(nk):
                    nc.tensor.transpose(
                        ps_w[:, k2 * T_core : (k2 + 1) * T_core],
                        w_sw[:, k2 * P : (k2 + 1) * P],
                        ident_f[:T_core, :T_core],
                    )
                wT = sb.tile([P, 2 * T_core], f32, name=f"wT{b}", tag="wT")
                nc.vector.tensor_copy(wT[:, : nk * T_core], ps_w[:, : nk * T_core])
                ps_attn = psp.tile([T_core, H], f32, name=f"attn_ps{b}", tag="mmC", bufs=2)
                for k2 in range(nk):
                    en, r2 = enc_nat[k2]
                    nc.tensor.matmul(
                        ps_attn,
                        wT[:r2, k2 * T_core : (k2 + 1) * T_core],
                        en[:r2, :],
                        start=(k2 == 0),
                        stop=(k2 == nk - 1),
                    )
                out_sb = sb.tile([T_core, H], f32, name=f"out{b}", tag="outsb")
                nc.vector.tensor_scalar_mul(out_sb, ps_attn, recip)
                nc.sync.dma_start(out_d[b], out_sb)

            def batch_loop():
                # software pipeline: load + phase A of batch b+1 are emitted
                # ahead of the heavy phase B/C of batch b.
                st = load(0)
                pa = phase_a(0, *st)
                for b in range(B):
                    if b + 1 < B:
                        nxt_st = load(b + 1)
                        nxt_pa = phase_a(b + 1, *nxt_st)
                    phase_bc(b, st[0], *pa)
                    if b + 1 < B:
                        st, pa = nxt_st, nxt_pa

            if reps > 1:
                with tc.For_i(0, reps, 1):
                    batch_loop()
            else:
                batch_loop()

    nc.compile()
    return nc


LAST_EXEC_NS = None


def _prep_weights(W_h, W_s, v, H):
    """gamma-scaled, transposed, chunk-blocked bf16 weight layouts."""
    import ml_dtypes
    HC = H // P
    def blockT(W):
        # whT block k (cols [k*H,(k+1)*H)) = W^T[h' in chunk k, :]
        Wt = (GAMMA * W.astype(np.float64)).T.astype(np.float32)  # [h', h]
        return np.ascontiguousarray(
            Wt.reshape(HC, P, H).transpose(1, 0, 2).reshape(P, HC * H)
        ).astype(ml_dtypes.bfloat16)
    v32 = np.zeros((P, HC * 32), np.float32)
    for c in range(HC):
        v32[:, c * 32] = v[c * P : (c + 1) * P]
    return blockT(W_h), blockT(W_s), v32.astype(ml_dtypes.bfloat16)


def _get_program(key):
    B, T_core, S, H, L, Lh = key
    return _build_program(B, T_core, S, H, list(L), list(Lh))


def kernel(query, encoder_outputs, src_lengths, W_h, W_s, v):
    global LAST_EXEC_NS
    from concourse.bass_utils import run_bass_kernel_spmd

    query = np.ascontiguousarray(np.asarray(query, dtype=np.float32))
    enc = np.ascontiguousarray(np.asarray(encoder_outputs, dtype=np.float32))
    W_h = np.ascontiguousarray(np.asarray(W_h, dtype=np.float32))
    W_s = np.ascontiguousarray(np.asarray(W_s, dtype=np.float32))
    v = np.ascontiguousarray(np.asarray(v, dtype=np.float32)).reshape(-1)
    L = [int(x) for x in np.asarray(src_lengths).reshape(-1)]

    B, T, H = query.shape
    S = enc.shape[1]
    T_core = T // NCORES
    Lh = [min(S, ((l + 3) // 4) * 4) for l in L]

    nc = _get_program((B, T_core, S, H, tuple(L), tuple(Lh)))

    whT, wsT, v32 = _prep_weights(W_h, W_s, v, H)
    in_maps = [
        {
            "enc": enc,
            "q": np.ascontiguousarray(query[:, i * T_core : (i + 1) * T_core, :]),
            "whT": whT,
            "wsT": wsT,
            "v32": v32,
        }
        for i in range(NCORES)
    ]
    res = run_bass_kernel_spmd(nc, in_maps, list(range(NCORES)))
    LAST_EXEC_NS = res.exec_time_ns
    out = np.concatenate([res.results[i]["out"] for i in range(NCORES)], axis=1)
    return out
